# revision 15
# baseline (speedup 1.0000x reference)
"""Causal self-attention (B=4, N=2048, D=1024, H=16) on 8 trn2 NeuronCores.

Sharding: core c -> (batch b = c//2, head-group hg = c%2).  Each core runs
one batch with 8 of the 16 heads; host adds the two head-group partials.

v2 layout (all matmuls lhsT.T @ rhs, PSUM f32):
  - Stage 1 runs fully in bf16 (x^T and W tiles bf16): Q^T/K^T come out
    directly (lhsT = W, rhs = X^T), V in natural [seq, head*hd] layout.
  - Attention processes HEAD PAIRS (hh=0/1): the two K=64 score matmuls
    use PE row groups 0-63 / 64-127 (tile_position auto-derived from the
    base partitions) so they run concurrently, and both heads' scores
    land in one 2-bank [128, 1024] PSUM tile -> ONE exp instruction per
    j-tile covers both heads (halves ACT instruction overhead).
  - Exact-causal trimming: diagonal-band j-tiles compute scores / exp /
    mask / P@V only on the valid i >= j region (dual-range strided APs
    for exp/mask across the two head halves).
  - Softmax denominator rides as a ones-column in V (row 64 of the P@V
    psum).  Normalization: DVE evicts pv into the aot rows early (frees
    the PSUM banks), reciprocal per head, one K=33 "L-matrix" matmul
    broadcasts both heads' recips across partitions, and the final
    in-place muls run on GpSimd (Pool engine, otherwise idle).
  - The {0,1} causal masks multiply on DVE, restricted to the 128-wide
    diagonal block where the mask is non-trivial (beyond it, all-ones).
  - Phase software-pipelining: stage 1 of chunk ic+1 and the output
    projection of chunk ic are emitted AFTER attention(ic), so their PE
    work fills the bubbles of the ACT-bound attention inner loop.
"""

import numpy as np
import ml_dtypes

import concourse.bass as bass
import concourse.mybir as mybir
import concourse.tile as tile
from concourse import bacc
from concourse.bass_utils import run_bass_kernel_spmd

AF = mybir.ActivationFunctionType
F32 = mybir.dt.float32
F32R = mybir.dt.float32r
BF16 = mybir.dt.bfloat16

B, N, D = 4, 2048, 1024
H, HD = 16, 64
HG = 8                 # heads per core
C = HG * HD            # 512: per-core head width
NT = N // 128          # 16 seq tiles
KT = D // 128          # 8 contraction tiles of the x->qkv matmul
IC = N // 512          # 4 query chunks
NP = HG // 2           # 4 head pairs
SCALE = HD ** -0.5
N_CORES = 8


def build_nc(reps=1, skip_attn=False, variant=""):
    nc = bacc.Bacc("TRN2", target_bir_lowering=False, debug=False,
                   num_devices=N_CORES)

    xT = nc.dram_tensor("xT", [D, N], BF16, kind="ExternalInput")
    wq = nc.dram_tensor("wq", [D, C], BF16, kind="ExternalInput")
    wk = nc.dram_tensor("wk", [D, C], BF16, kind="ExternalInput")
    wv = nc.dram_tensor("wv", [D, C], BF16, kind="ExternalInput")
    bq = nc.dram_tensor("bq", [C, 1], F32, kind="ExternalInput")
    wp = nc.dram_tensor("wp", [C, D], BF16, kind="ExternalInput")
    bp = nc.dram_tensor("bp", [128, D], F32, kind="ExternalInput")
    tm = nc.dram_tensor("tm", [128, 2048], BF16, kind="ExternalInput")
    out = nc.dram_tensor("out", [N, D], F32, kind="ExternalOutput")

    with tile.TileContext(nc) as tc:
        with (
            tc.tile_pool(name="persist", bufs=1) as persist,
            tc.tile_pool(name="xt", bufs=2) as xt_pool,
            tc.tile_pool(name="qt", bufs=2) as qt_pool,
            tc.tile_pool(name="aot", bufs=2) as aot_pool,
            tc.tile_pool(name="pt", bufs=6) as pt_pool,
            tc.tile_pool(name="ostage", bufs=4) as ostage_pool,
            tc.tile_pool(name="small", bufs=4) as small_pool,
            tc.tile_pool(name="ps_sc", bufs=(4 if variant == "seqsc" else 2),
                         space="PSUM") as ps_sc,
            tc.tile_pool(name="ps_pv", bufs=2, space="PSUM") as ps_pv,
            tc.tile_pool(name="ps_mm", bufs=2, space="PSUM") as ps_mm,
        ):
            # ---- persistent SBUF tensors ----
            # DMA priority order: first stage-1 group needs wq + x^T chunk0
            # + bq, so those go first; attention-only tensors come last.
            # One dma_start per tensor: a single InstDMACopy is split across
            # all 16 SDMA engines (near-full HBM rate) and costs one issue
            # slot on the sync queue instead of eight.
            wq_sb = persist.tile([128, KT, C], BF16)
            wk_sb = persist.tile([128, KT, C], BF16)
            wv_sb = persist.tile([128, KT, C], BF16)

            def dma_xt_chunk(ic):
                xt_t = xt_pool.tile([128, KT, 512], BF16, name="xt")
                nc.sync.dma_start(
                    out=xt_t[:, :, :],
                    in_=xT.rearrange("(t p) n -> p t n", p=128)[
                        :, :, ic * 512:(ic + 1) * 512],
                )
                return xt_t

            nc.sync.dma_start(out=wq_sb[:, :, :],
                              in_=wq.rearrange("(t p) c -> p t c", p=128))
            bq_sb = persist.tile([128, C // 128], F32)
            nc.sync.dma_start(out=bq_sb[:, :],
                              in_=bq.rearrange("(t p) o -> p (t o)", p=128))
            xt_first = dma_xt_chunk(0)
            nc.sync.dma_start(out=wk_sb[:, :, :],
                              in_=wk.rearrange("(t p) c -> p t c", p=128))
            nc.sync.dma_start(out=wv_sb[:, :, :],
                              in_=wv.rearrange("(t p) c -> p t c", p=128))
            tm_sb = persist.tile([128, 2048], BF16)
            nc.sync.dma_start(out=tm_sb[:, :], in_=tm[:, :])
            wp_sb = persist.tile([128, C // 128, D], BF16)
            nc.sync.dma_start(out=wp_sb[:, :, :],
                              in_=wp.rearrange("(c p) d -> p c d", p=128))
            bp_sb = persist.tile([128, D], F32)
            nc.sync.dma_start(out=bp_sb[:, :], in_=bp[:, :])



            kt_sb = persist.tile([128, C // 128, N], BF16)   # K^T, c-major
            v_sb = persist.tile([128, NT, HG, HD + 1], BF16)  # V + ones col
            nc.gpsimd.memset(v_sb[:, :, :, HD:HD + 1], 1.0)

            def stage1(ic, xt_t):
                """QKV projection for query chunk ic."""
                qt_t = qt_pool.tile([128, C // 128, 512], BF16, name="qt")
                groups = []
                for ct in range(C // 128):
                    groups.append((
                        lambda p, ct=ct: [nc.tensor.matmul(
                            p, wq_sb[:, kt, ct * 128:(ct + 1) * 128],
                            xt_t[:, kt, :],
                            start=(kt == 0), stop=(kt == KT - 1))
                            for kt in range(KT)],
                        lambda p, ct=ct: nc.vector.tensor_scalar_add(
                            qt_t[:, ct, :], p, bq_sb[:, ct:ct + 1]),
                    ))
                for ct in range(C // 128):
                    groups.append((
                        lambda p, ct=ct: [nc.tensor.matmul(
                            p, wk_sb[:, kt, ct * 128:(ct + 1) * 128],
                            xt_t[:, kt, :],
                            start=(kt == 0), stop=(kt == KT - 1))
                            for kt in range(KT)],
                        lambda p, ct=ct: nc.vector.tensor_copy(
                            kt_sb[:, ct, ic * 512:(ic + 1) * 512], p),
                    ))
                for ntl in range(4):
                    jt = ic * 4 + ntl
                    groups.append((
                        lambda p, ntl=ntl: [nc.tensor.matmul(
                            p, xt_t[:, kt, ntl * 128:(ntl + 1) * 128],
                            wv_sb[:, kt, :],
                            start=(kt == 0), stop=(kt == KT - 1))
                            for kt in range(KT)],
                        lambda p, jt=jt: nc.vector.tensor_copy(
                            v_sb[:, jt, :, 0:HD],
                            p.rearrange("p (h w) -> p h w", w=HD)),
                    ))
                for emit_mms, evict in groups:
                    sl = ps_mm.tile([128, 512], F32, name="mm")
                    emit_mms(sl[:, :])
                    evict(sl[:, :])
                return qt_t

            def attention_pair(ic, p, qt_t, aot_t):
                """Scores+softmax+P@V for head pair p of chunk ic.
                Returns a closure that emits the normalization tail."""
                njt = 4 * ic + 4
                h0, h1 = 2 * p, 2 * p + 1
                pv0 = ps_pv.tile([HD + 1, 512], F32, name="pv")
                pv1 = ps_pv.tile([HD + 1, 512], F32, name="pv")
                pts = [None] * njt   # (pt_tile, lo) per j-tile

                def sc_stage(jt):
                    t = jt - 4 * ic          # >=0 on the diagonal band
                    lo = max(t, 0) * 128     # first valid query col
                    if variant == "seqsc":
                        # one 1-bank psum tile + dense exp + 1D mask per
                        # head half, finer sc->exp pipelining
                        pt = pt_pool.tile([128, 1024], BF16, name="pt")
                        m = tm_sb.rearrange("q (g w) -> q g w", g=4)
                        for hh in range(2):
                            ps = ps_sc.tile([128, 512], F32, name="sc")
                            nc.tensor.matmul(
                                ps[:, lo:512],
                                kt_sb[hh * 64:hh * 64 + 64, p,
                                      jt * 128:(jt + 1) * 128],
                                qt_t[hh * 64:hh * 64 + 64, p, lo:512],
                                start=True, stop=True)
                            o = 512 * hh
                            nc.scalar.activation(
                                pt[:, o + lo:o + 512], ps[:, lo:512],
                                AF.Exp, scale=SCALE)
                            if t >= 0 and variant != "nomask":
                                nc.vector.tensor_mul(
                                    pt[:, o + lo:o + lo + 128],
                                    pt[:, o + lo:o + lo + 128],
                                    m[:, t, lo:lo + 128])
                        pts[jt] = (pt, lo)
                        return
                    ps = ps_sc.tile([128, 1024], F32, name="sc")
                    nc.tensor.matmul(
                        ps[:, lo:512],
                        kt_sb[0:64, p, jt * 128:(jt + 1) * 128],
                        qt_t[0:64, p, lo:512],
                        start=True, stop=True)
                    nc.tensor.matmul(
                        ps[:, 512 + lo:1024],
                        kt_sb[64:128, p, jt * 128:(jt + 1) * 128],
                        qt_t[64:128, p, lo:512],
                        start=True, stop=True)
                    pt = pt_pool.tile([128, 1024], BF16, name="pt")
                    if variant == "splitexp":
                        # dense 1D APs, one exp per head half
                        nc.scalar.activation(pt[:, lo:512], ps[:, lo:512],
                                             AF.Exp, scale=SCALE)
                        nc.scalar.activation(pt[:, 512 + lo:1024],
                                             ps[:, 512 + lo:1024],
                                             AF.Exp, scale=SCALE)
                    elif lo:
                        ap_in = ps.rearrange(
                            "q (g w) -> q g w", g=2)[:, :, lo:512]
                        ap_out = pt.rearrange(
                            "q (g w) -> q g w", g=2)[:, :, lo:512]
                        nc.scalar.activation(ap_out, ap_in, AF.Exp,
                                             scale=SCALE)
                    else:
                        nc.scalar.activation(pt[:, :], ps[:, :], AF.Exp,
                                             scale=SCALE)
                    if t >= 0 and variant != "nomask":
                        # only the 128-wide diagonal block i in
                        # [128t, 128t+128) has non-trivial mask values;
                        # everything beyond it is all-ones
                        m = tm_sb.rearrange("q (g w) -> q g w", g=4)
                        eng = nc.gpsimd if variant == "gmask" else nc.vector
                        if variant == "splitexp":
                            eng.tensor_mul(
                                pt[:, lo:lo + 128], pt[:, lo:lo + 128],
                                m[:, t, lo:lo + 128])
                            eng.tensor_mul(
                                pt[:, 512 + lo:512 + lo + 128],
                                pt[:, 512 + lo:512 + lo + 128],
                                m[:, t, lo:lo + 128])
                        else:
                            d_out = pt.rearrange(
                                "q (g w) -> q g w", g=2)[:, :, lo:lo + 128]
                            eng.tensor_mul(
                                d_out, d_out,
                                m[:, t:t + 1, lo:lo + 128].broadcast_to(
                                    (128, 2, 128)))
                    pts[jt] = (pt, lo)

                def pv_stage(jt):
                    pt, lo = pts[jt]
                    nc.tensor.matmul(
                        pv0[:, lo:512], v_sb[:, jt, h0, :], pt[:, lo:512],
                        start=(jt == 0), stop=(jt == njt - 1))
                    nc.tensor.matmul(
                        pv1[:, lo:512], v_sb[:, jt, h1, :],
                        pt[:, 512 + lo:1024],
                        start=(jt == 0), stop=(jt == njt - 1))
                    pts[jt] = None

                LAG = 3
                for jt in range(njt):
                    sc_stage(jt)
                    if jt >= LAG:
                        pv_stage(jt - LAG)
                for jt in range(max(0, njt - LAG), njt):
                    pv_stage(jt)

                # evict unnormalized attn-out straight into the aot rows
                # (DVE allows the 0->64 partition shift) and take the two
                # denominator recips; pv banks free right after
                den = small_pool.tile([1, 1024], F32, name="den",
                                      tag="den", bufs=3)
                rcf = small_pool.tile([1, 1024], F32, name="rcf",
                                      tag="rcf", bufs=3)
                if variant != "noepi":
                    nc.vector.tensor_copy(den[0:1, 0:512],
                                          pv0[HD:HD + 1, :])
                    nc.vector.tensor_copy(den[0:1, 512:1024],
                                          pv1[HD:HD + 1, :])
                nc.vector.tensor_copy(aot_t[0:64, p, :], pv0[0:HD, :])
                nc.vector.tensor_copy(aot_t[64:128, p, :], pv1[0:HD, :])
                if variant != "noepi":
                    # ~5x faster than nc.vector.reciprocal; denominators are
                    # in [~1e-2, ~1e6] so the approx edge cases can't hit,
                    # and ~18 correct bits is far beyond what we need.
                    # The custom DVE op needs SBUF in/out at partition 0
                    # with plain-copy staging around it: cross-engine deps
                    # on its APs are unreliable (silent no-op / garbage),
                    # so every producer/consumer it touches stays on DVE
                    # where FIFO order protects it.
                    nc.vector.reciprocal_approx_fast(
                        out=rcf[0:1, :], in_=den[0:1, :])

                def tail():
                    # All-gpsimd normalization: stage the recips to bf16 on
                    # gpsimd, partition-broadcast each head's row to a full
                    # [128, 512] tile (the op only supports partition-0 in /
                    # partition-0-based out), then two half-range in-place
                    # muls.  No PE, no PSUM, no DVE involvement.
                    if variant == "noepi":
                        return
                    rcb = small_pool.tile([1, 1024], BF16, name="rcb",
                                          tag="rcb", bufs=2)
                    repa = small_pool.tile([128, 512], BF16, name="repa",
                                           tag="repa", bufs=2)
                    repb = small_pool.tile([128, 512], BF16, name="repb",
                                           tag="repb", bufs=2)
                    nc.gpsimd.tensor_copy(rcb[0:1, :], rcf[0:1, :])
                    nc.gpsimd.partition_broadcast(repa[:, :],
                                                  rcb[0:1, 0:512])
                    nc.gpsimd.partition_broadcast(repb[:, :],
                                                  rcb[0:1, 512:1024])
                    nc.gpsimd.tensor_mul(
                        aot_t[0:64, p, :], aot_t[0:64, p, :], repa[0:64, :])
                    nc.gpsimd.tensor_mul(
                        aot_t[64:128, p, :], aot_t[64:128, p, :],
                        repb[64:128, :])
                return tail

            def attention(ic, qt_t):
                # tails deferred by TWO pairs so the rep matmul's rc2 input
                # (produced at the end of the DVE queue for its pair) is
                # ready long before the PE reaches the tail in FIFO order
                aot_t = aot_pool.tile([128, C // 128, 512], BF16, name="aot")
                pending = []
                for p in range(0 if skip_attn else NP):
                    t = attention_pair(ic, p, qt_t, aot_t)
                    pending.append(t)
                    if len(pending) > 2:
                        pending.pop(0)()
                for t in pending:
                    t()
                return aot_t

            def proj(ic, aot_t):
                for ntl in range(4):
                    nt = ic * 4 + ntl
                    pss = [ps_mm.tile([128, 512], F32, name="mm")
                           for _ in range(2)]
                    for ct in range(C // 128):
                        for dc in range(2):
                            nc.tensor.matmul(
                                pss[dc][:, :],
                                aot_t[:, ct, ntl * 128:(ntl + 1) * 128],
                                wp_sb[:, ct, dc * 512:(dc + 1) * 512],
                                start=(ct == 0), stop=(ct == C // 128 - 1),
                            )
                    ot = ostage_pool.tile([128, 1024], F32, name="ot")
                    for dc in range(2):
                        nc.vector.tensor_add(
                            ot[:, dc * 512:(dc + 1) * 512], pss[dc][:, :],
                            bp_sb[:, dc * 512:(dc + 1) * 512])
                    nc.sync.dma_start(
                        out=out[nt * 128:(nt + 1) * 128, :], in_=ot[:, :])

            for _rep in range(reps):
                qt = None
                xt_next = xt_first if _rep == 0 else dma_xt_chunk(0)
                for ic in range(IC):
                    if ic == 0:
                        qt = stage1(0, xt_next)
                    aot_t = attention(ic, qt)
                    if ic + 1 < IC:
                        xt_next = dma_xt_chunk(ic + 1)
                        qt = stage1(ic + 1, xt_next)
                    proj(ic, aot_t)

    nc.compile()
    return nc


_NC = None


def _get_nc():
    global _NC
    if _NC is None:
        _NC = build_nc()
    return _NC


def _make_tri_masks():
    pj = np.arange(128)[:, None]
    fi = np.arange(512)[None, :]
    blocks = [(fi >= 128 * t + pj) for t in range(4)]
    return np.concatenate(blocks, axis=1).astype(ml_dtypes.bfloat16)


def _numpy_reference(x, causal_mask, Wqkv, bqkv, Wproj, bproj):
    b, n, d = x.shape
    qkv = x @ Wqkv + bqkv
    qkv = qkv.reshape(b, n, 3, H, HD).transpose(2, 0, 3, 1, 4)
    q, k, v = qkv[0], qkv[1], qkv[2]
    s = np.einsum("bhqd,bhkd->bhqk", q, k) * (HD ** -0.5) + causal_mask
    s = s - s.max(axis=-1, keepdims=True)
    p = np.exp(s)
    p /= p.sum(axis=-1, keepdims=True)
    o = np.einsum("bhqk,bhkd->bhqd", p, v)
    o = o.transpose(0, 2, 1, 3).reshape(b, n, d)
    return (o @ Wproj + bproj).astype(np.float32)


def build_in_maps(inputs):
    x = np.asarray(inputs["x"], dtype=np.float32)
    Wqkv = np.asarray(inputs["Wqkv"], dtype=np.float32)
    bqkv = np.asarray(inputs["bqkv"], dtype=np.float32)
    Wproj = np.asarray(inputs["Wproj"], dtype=np.float32)
    bproj = np.asarray(inputs["bproj"], dtype=np.float32)
    tmask = _make_tri_masks()
    bf = ml_dtypes.bfloat16
    xTs = [np.ascontiguousarray(x[b].T.astype(bf)) for b in range(B)]
    in_maps = []
    for c in range(N_CORES):
        b, hg = c // 2, c % 2
        cs = slice(hg * C, (hg + 1) * C)
        wp_rows = Wproj[hg * C:(hg + 1) * C, :]
        bv = bqkv[2 * D + hg * C: 2 * D + (hg + 1) * C]
        bp_row = bv @ wp_rows + (bproj if hg == 0 else 0.0)
        in_maps.append({
            "xT": xTs[b],
            "wq": np.ascontiguousarray(Wqkv[:, cs].astype(bf)),
            "wk": np.ascontiguousarray(
                Wqkv[:, D + hg * C: D + (hg + 1) * C].astype(bf)),
            "wv": np.ascontiguousarray(
                Wqkv[:, 2 * D + hg * C: 2 * D + (hg + 1) * C].astype(bf)),
            "bq": np.ascontiguousarray(bqkv[cs].reshape(C, 1)),
            "wp": np.ascontiguousarray(wp_rows.astype(bf)),
            "bp": np.ascontiguousarray(
                np.broadcast_to(bp_row.astype(np.float32), (128, D))),
            "tm": tmask,
        })
    return in_maps


def kernel(x, causal_mask, Wqkv, bqkv, Wproj, bproj):
    x = np.asarray(x, dtype=np.float32)
    causal_mask = np.asarray(causal_mask, dtype=np.float32)
    Wqkv = np.asarray(Wqkv, dtype=np.float32)
    bqkv = np.asarray(bqkv, dtype=np.float32)
    Wproj = np.asarray(Wproj, dtype=np.float32)
    bproj = np.asarray(bproj, dtype=np.float32)

    # the device kernel applies causality structurally; verify the provided
    # mask is the standard causal mask and fall back to numpy if it isn't
    expected_mask = np.where(
        np.triu(np.ones((N, N), dtype=bool), k=1),
        np.float32(-1e9), np.float32(0.0))
    if causal_mask.shape != (N, N) or not np.array_equal(
            causal_mask, expected_mask):
        return _numpy_reference(x, causal_mask, Wqkv, bqkv, Wproj, bproj)

    nc = _get_nc()
    in_maps = build_in_maps(
        dict(x=x, Wqkv=Wqkv, bqkv=bqkv, Wproj=Wproj, bproj=bproj))

    res = run_bass_kernel_spmd(nc, in_maps, core_ids=list(range(N_CORES)))
    outs = [r["out"] for r in res.results]
    return np.stack([outs[2 * b] + outs[2 * b + 1] for b in range(B)], axis=0)



# revision 18
# speedup vs baseline: 1.3527x; 1.3527x over previous
"""Causal self-attention (B=4, N=2048, D=1024, H=16) on 8 trn2 NeuronCores.

Sharding: core c -> (batch b = c//2, head-group hg = c%2).  Each core runs
one batch with 8 of the 16 heads; host adds the two head-group partials.

v2 layout (all matmuls lhsT.T @ rhs, PSUM f32):
  - Stage 1 runs fully in bf16 (x^T and W tiles bf16): Q^T/K^T come out
    directly (lhsT = W, rhs = X^T), V in natural [seq, head*hd] layout.
  - Attention processes HEAD PAIRS (hh=0/1): the two K=64 score matmuls
    use PE row groups 0-63 / 64-127 (tile_position auto-derived from the
    base partitions) so they run concurrently, and both heads' scores
    land in one 2-bank [128, 1024] PSUM tile -> ONE exp instruction per
    j-tile covers both heads (halves ACT instruction overhead).
  - Exact-causal trimming: diagonal-band j-tiles compute scores / exp /
    mask / P@V only on the valid i >= j region (dual-range strided APs
    for exp/mask across the two head halves).
  - Softmax denominator rides as a ones-column in V (row 64 of the P@V
    psum).  Normalization: DVE evicts pv into the aot rows early (frees
    the PSUM banks), reciprocal per head, one K=33 "L-matrix" matmul
    broadcasts both heads' recips across partitions, and the final
    in-place muls run on GpSimd (Pool engine, otherwise idle).
  - The {0,1} causal masks multiply on DVE, restricted to the 128-wide
    diagonal block where the mask is non-trivial (beyond it, all-ones).
  - Phase software-pipelining: stage 1 of chunk ic+1 and the output
    projection of chunk ic are emitted AFTER attention(ic), so their PE
    work fills the bubbles of the ACT-bound attention inner loop.
"""

import numpy as np
import ml_dtypes

import concourse.bass as bass
import concourse.mybir as mybir
import concourse.tile as tile
from concourse import bacc
from concourse.bass_utils import run_bass_kernel_spmd

AF = mybir.ActivationFunctionType
F32 = mybir.dt.float32
F32R = mybir.dt.float32r
BF16 = mybir.dt.bfloat16

B, N, D = 4, 2048, 1024
H, HD = 16, 64
HG = 8                 # heads per core
C = HG * HD            # 512: per-core head width
NT = N // 128          # 16 seq tiles
KT = D // 128          # 8 contraction tiles of the x->qkv matmul
IC = N // 512          # 4 query chunks
NP = HG // 2           # 4 head pairs
SCALE = HD ** -0.5
N_CORES = 8


def build_nc(reps=1, skip_attn=False, variant=""):
    nc = bacc.Bacc("TRN2", target_bir_lowering=False, debug=False,
                   num_devices=N_CORES)

    xT = nc.dram_tensor("xT", [D, N], BF16, kind="ExternalInput")
    wq = nc.dram_tensor("wq", [D, C], BF16, kind="ExternalInput")
    wk = nc.dram_tensor("wk", [D, C], BF16, kind="ExternalInput")
    wv = nc.dram_tensor("wv", [D, C], BF16, kind="ExternalInput")
    bq = nc.dram_tensor("bq", [C, 1], F32, kind="ExternalInput")
    wp = nc.dram_tensor("wp", [C, D], BF16, kind="ExternalInput")
    bp = nc.dram_tensor("bp", [128, D], F32, kind="ExternalInput")
    tm = nc.dram_tensor("tm", [128, 2048], BF16, kind="ExternalInput")
    out = nc.dram_tensor("out", [N, D], F32, kind="ExternalOutput")

    with tile.TileContext(nc) as tc:
        with (
            tc.tile_pool(name="persist", bufs=1) as persist,
            tc.tile_pool(name="xt", bufs=2) as xt_pool,
            tc.tile_pool(name="qt", bufs=2) as qt_pool,
            tc.tile_pool(name="aot", bufs=2) as aot_pool,
            tc.tile_pool(name="pt", bufs=6) as pt_pool,
            tc.tile_pool(name="ostage", bufs=4) as ostage_pool,
            tc.tile_pool(name="small", bufs=4) as small_pool,
            tc.tile_pool(name="ps_sc", bufs=(4 if variant == "seqsc" else 2),
                         space="PSUM") as ps_sc,
            tc.tile_pool(name="ps_pv", bufs=2, space="PSUM") as ps_pv,
            tc.tile_pool(name="ps_mm", bufs=2, space="PSUM") as ps_mm,
        ):
            # ---- persistent SBUF tensors ----
            # DMA priority order: first stage-1 group needs wq + x^T chunk0
            # + bq, so those go first; attention-only tensors come last.
            # One dma_start per tensor: a single InstDMACopy is split across
            # all 16 SDMA engines (near-full HBM rate) and costs one issue
            # slot on the sync queue instead of eight.
            wq_sb = persist.tile([128, KT, C], BF16)
            wk_sb = persist.tile([128, KT, C], BF16)
            wv_sb = persist.tile([128, KT, C], BF16)

            def dma_xt_chunk(ic):
                xt_t = xt_pool.tile([128, KT, 512], BF16, name="xt")
                nc.sync.dma_start(
                    out=xt_t[:, :, :],
                    in_=xT.rearrange("(t p) n -> p t n", p=128)[
                        :, :, ic * 512:(ic + 1) * 512],
                )
                return xt_t

            nc.sync.dma_start(out=wq_sb[:, :, :],
                              in_=wq.rearrange("(t p) c -> p t c", p=128))
            bq_sb = persist.tile([128, C // 128], F32)
            nc.sync.dma_start(out=bq_sb[:, :],
                              in_=bq.rearrange("(t p) o -> p (t o)", p=128))
            xt_first = dma_xt_chunk(0)
            nc.sync.dma_start(out=wk_sb[:, :, :],
                              in_=wk.rearrange("(t p) c -> p t c", p=128))
            nc.sync.dma_start(out=wv_sb[:, :, :],
                              in_=wv.rearrange("(t p) c -> p t c", p=128))
            tm_sb = persist.tile([128, 2048], BF16)
            nc.sync.dma_start(out=tm_sb[:, :], in_=tm[:, :])
            wp_sb = persist.tile([128, C // 128, D], BF16)
            nc.sync.dma_start(out=wp_sb[:, :, :],
                              in_=wp.rearrange("(c p) d -> p c d", p=128))
            bp_sb = persist.tile([128, D], F32)
            nc.sync.dma_start(out=bp_sb[:, :], in_=bp[:, :])



            # L-matrix for the recip partition-broadcast: out rows 0:64 copy
            # recip row 0, rows 64:128 copy recip row 32 (K=33 matmul; the
            # zero rows 1..31 contribute nothing).  Row 32 because engine
            # APs must start at 32-aligned partitions.
            l2_f = persist.tile([33, 128], F32)
            nc.vector.memset(l2_f[:, :], 0.0)
            nc.vector.memset(l2_f[0:1, 0:64], 1.0)
            nc.vector.memset(l2_f[32:33, 64:128], 1.0)
            l2_sb = persist.tile([33, 128], F32R)
            nc.vector.tensor_copy(l2_sb[:, :], l2_f[:, :])

            kt_sb = persist.tile([128, C // 128, N], BF16)   # K^T, c-major
            v_sb = persist.tile([128, NT, HG, HD + 1], BF16)  # V + ones col
            nc.gpsimd.memset(v_sb[:, :, :, HD:HD + 1], 1.0)

            def stage1(ic, xt_t):
                """QKV projection for query chunk ic."""
                qt_t = qt_pool.tile([128, C // 128, 512], BF16, name="qt")
                groups = []
                for ct in range(C // 128):
                    groups.append((
                        lambda p, ct=ct: [nc.tensor.matmul(
                            p, wq_sb[:, kt, ct * 128:(ct + 1) * 128],
                            xt_t[:, kt, :],
                            start=(kt == 0), stop=(kt == KT - 1))
                            for kt in range(KT)],
                        lambda p, ct=ct: nc.vector.tensor_scalar_add(
                            qt_t[:, ct, :], p, bq_sb[:, ct:ct + 1]),
                    ))
                for ct in range(C // 128):
                    groups.append((
                        lambda p, ct=ct: [nc.tensor.matmul(
                            p, wk_sb[:, kt, ct * 128:(ct + 1) * 128],
                            xt_t[:, kt, :],
                            start=(kt == 0), stop=(kt == KT - 1))
                            for kt in range(KT)],
                        lambda p, ct=ct: nc.vector.tensor_copy(
                            kt_sb[:, ct, ic * 512:(ic + 1) * 512], p),
                    ))
                for ntl in range(4):
                    jt = ic * 4 + ntl
                    groups.append((
                        lambda p, ntl=ntl: [nc.tensor.matmul(
                            p, xt_t[:, kt, ntl * 128:(ntl + 1) * 128],
                            wv_sb[:, kt, :],
                            start=(kt == 0), stop=(kt == KT - 1))
                            for kt in range(KT)],
                        lambda p, jt=jt: nc.vector.tensor_copy(
                            v_sb[:, jt, :, 0:HD],
                            p.rearrange("p (h w) -> p h w", w=HD)),
                    ))
                for emit_mms, evict in groups:
                    sl = ps_mm.tile([128, 512], F32, name="mm")
                    emit_mms(sl[:, :])
                    evict(sl[:, :])
                return qt_t

            def attention_pair(ic, p, qt_t, aot_t):
                """Scores+softmax+P@V for head pair p of chunk ic.
                Returns a closure that emits the normalization tail."""
                njt = 4 * ic + 4
                h0, h1 = 2 * p, 2 * p + 1
                pv0 = ps_pv.tile([HD + 1, 512], F32, name="pv")
                pv1 = ps_pv.tile([HD + 1, 512], F32, name="pv")
                pts = [None] * njt   # (pt_tile, lo) per j-tile

                def sc_stage(jt):
                    t = jt - 4 * ic          # >=0 on the diagonal band
                    lo = max(t, 0) * 128     # first valid query col
                    if variant == "seqsc":
                        # one 1-bank psum tile + dense exp + 1D mask per
                        # head half, finer sc->exp pipelining
                        pt = pt_pool.tile([128, 1024], BF16, name="pt")
                        m = tm_sb.rearrange("q (g w) -> q g w", g=4)
                        for hh in range(2):
                            ps = ps_sc.tile([128, 512], F32, name="sc")
                            nc.tensor.matmul(
                                ps[:, lo:512],
                                kt_sb[hh * 64:hh * 64 + 64, p,
                                      jt * 128:(jt + 1) * 128],
                                qt_t[hh * 64:hh * 64 + 64, p, lo:512],
                                start=True, stop=True)
                            o = 512 * hh
                            nc.scalar.activation(
                                pt[:, o + lo:o + 512], ps[:, lo:512],
                                AF.Exp, scale=SCALE)
                            if t >= 0 and variant != "nomask":
                                nc.vector.tensor_mul(
                                    pt[:, o + lo:o + lo + 128],
                                    pt[:, o + lo:o + lo + 128],
                                    m[:, t, lo:lo + 128])
                        pts[jt] = (pt, lo)
                        return
                    ps = ps_sc.tile([128, 1024], F32, name="sc")
                    nc.tensor.matmul(
                        ps[:, lo:512],
                        kt_sb[0:64, p, jt * 128:(jt + 1) * 128],
                        qt_t[0:64, p, lo:512],
                        start=True, stop=True)
                    nc.tensor.matmul(
                        ps[:, 512 + lo:1024],
                        kt_sb[64:128, p, jt * 128:(jt + 1) * 128],
                        qt_t[64:128, p, lo:512],
                        start=True, stop=True)
                    pt = pt_pool.tile([128, 1024], BF16, name="pt")
                    if variant == "splitexp":
                        # dense 1D APs, one exp per head half
                        nc.scalar.activation(pt[:, lo:512], ps[:, lo:512],
                                             AF.Exp, scale=SCALE)
                        nc.scalar.activation(pt[:, 512 + lo:1024],
                                             ps[:, 512 + lo:1024],
                                             AF.Exp, scale=SCALE)
                    elif lo:
                        ap_in = ps.rearrange(
                            "q (g w) -> q g w", g=2)[:, :, lo:512]
                        ap_out = pt.rearrange(
                            "q (g w) -> q g w", g=2)[:, :, lo:512]
                        nc.scalar.activation(ap_out, ap_in, AF.Exp,
                                             scale=SCALE)
                    else:
                        nc.scalar.activation(pt[:, :], ps[:, :], AF.Exp,
                                             scale=SCALE)
                    if t >= 0 and variant != "nomask":
                        # only the 128-wide diagonal block i in
                        # [128t, 128t+128) has non-trivial mask values;
                        # everything beyond it is all-ones
                        m = tm_sb.rearrange("q (g w) -> q g w", g=4)
                        eng = nc.gpsimd if variant == "gmask" else nc.vector
                        if variant == "splitexp":
                            eng.tensor_mul(
                                pt[:, lo:lo + 128], pt[:, lo:lo + 128],
                                m[:, t, lo:lo + 128])
                            eng.tensor_mul(
                                pt[:, 512 + lo:512 + lo + 128],
                                pt[:, 512 + lo:512 + lo + 128],
                                m[:, t, lo:lo + 128])
                        else:
                            d_out = pt.rearrange(
                                "q (g w) -> q g w", g=2)[:, :, lo:lo + 128]
                            eng.tensor_mul(
                                d_out, d_out,
                                m[:, t:t + 1, lo:lo + 128].broadcast_to(
                                    (128, 2, 128)))
                    pts[jt] = (pt, lo)

                def pv_stage(jt):
                    pt, lo = pts[jt]
                    nc.tensor.matmul(
                        pv0[:, lo:512], v_sb[:, jt, h0, :], pt[:, lo:512],
                        start=(jt == 0), stop=(jt == njt - 1))
                    nc.tensor.matmul(
                        pv1[:, lo:512], v_sb[:, jt, h1, :],
                        pt[:, 512 + lo:1024],
                        start=(jt == 0), stop=(jt == njt - 1))
                    pts[jt] = None

                LAG = 3
                for jt in range(njt):
                    sc_stage(jt)
                    if jt >= LAG:
                        pv_stage(jt - LAG)
                for jt in range(max(0, njt - LAG), njt):
                    pv_stage(jt)

                # evict unnormalized attn-out straight into the aot rows
                # (DVE allows the 0->64 partition shift) and take the two
                # denominator recips; pv banks free right after
                den = small_pool.tile([1, 1024], F32, name="den",
                                      tag="den", bufs=3)
                rcf = small_pool.tile([1, 1024], F32, name="rcf",
                                      tag="rcf", bufs=3)
                if variant != "noepi":
                    nc.vector.tensor_copy(den[0:1, 0:512],
                                          pv0[HD:HD + 1, :])
                    nc.vector.tensor_copy(den[0:1, 512:1024],
                                          pv1[HD:HD + 1, :])
                nc.vector.tensor_copy(aot_t[0:64, p, :], pv0[0:HD, :])
                nc.vector.tensor_copy(aot_t[64:128, p, :], pv1[0:HD, :])
                if variant != "noepi":
                    # ~5x faster than nc.vector.reciprocal; denominators are
                    # in [~1e-2, ~1e6] so the approx edge cases can't hit,
                    # and ~18 correct bits is far beyond what we need.
                    # The custom DVE op needs SBUF in/out at partition 0
                    # with plain-copy staging around it: cross-engine deps
                    # on its APs are unreliable (silent no-op / garbage),
                    # so every producer/consumer it touches stays on DVE
                    # where FIFO order protects it.
                    nc.vector.reciprocal_approx_fast(
                        out=rcf[0:1, :], in_=den[0:1, :])

                rc2 = small_pool.tile([33, 512], F32R, name="rc",
                                      tag="rc", bufs=3)
                if variant != "noepi":
                    with nc.allow_low_precision(reason="f32r recip ok"):
                        nc.vector.tensor_copy(rc2[0:1, :], rcf[0:1, 0:512])
                        nc.vector.tensor_copy(rc2[32:33, :],
                                              rcf[0:1, 512:1024])

                def tail():
                    if variant == "noepi":
                        return
                    rep = ps_pv.tile([128, 512], F32, name="pv")
                    nc.tensor.matmul(rep[:, :], l2_sb[:, :], rc2[:, :],
                                     start=True, stop=True)
                    rep_sb = small_pool.tile([128, 512], BF16, name="repsb",
                                             tag="repsb", bufs=2)
                    nc.vector.tensor_copy(rep_sb[:, :], rep[:, :])
                    eng = nc.vector if variant == "dvemul" else nc.gpsimd
                    eng.tensor_mul(
                        aot_t[:, p, :], aot_t[:, p, :], rep_sb[:, :])
                return tail

            def attention(ic, qt_t):
                aot_t = aot_pool.tile([128, C // 128, 512], BF16, name="aot")
                pending = None
                for p in range(0 if skip_attn else NP):
                    t = attention_pair(ic, p, qt_t, aot_t)
                    if pending is not None:
                        pending()
                    pending = t
                if pending is not None:
                    pending()
                return aot_t

            def proj(ic, aot_t):
                for ntl in range(4):
                    nt = ic * 4 + ntl
                    pss = [ps_mm.tile([128, 512], F32, name="mm")
                           for _ in range(2)]
                    for ct in range(C // 128):
                        for dc in range(2):
                            nc.tensor.matmul(
                                pss[dc][:, :],
                                aot_t[:, ct, ntl * 128:(ntl + 1) * 128],
                                wp_sb[:, ct, dc * 512:(dc + 1) * 512],
                                start=(ct == 0), stop=(ct == C // 128 - 1),
                            )
                    ot = ostage_pool.tile([128, 1024], F32, name="ot")
                    for dc in range(2):
                        nc.vector.tensor_add(
                            ot[:, dc * 512:(dc + 1) * 512], pss[dc][:, :],
                            bp_sb[:, dc * 512:(dc + 1) * 512])
                    nc.sync.dma_start(
                        out=out[nt * 128:(nt + 1) * 128, :], in_=ot[:, :])

            for _rep in range(reps):
                qt = None
                xt_next = xt_first if _rep == 0 else dma_xt_chunk(0)
                for ic in range(IC):
                    if ic == 0:
                        qt = stage1(0, xt_next)
                    aot_t = attention(ic, qt)
                    if ic + 1 < IC:
                        xt_next = dma_xt_chunk(ic + 1)
                        qt = stage1(ic + 1, xt_next)
                    proj(ic, aot_t)

    nc.compile()
    return nc


_NC = None


def _get_nc():
    global _NC
    if _NC is None:
        _NC = build_nc()
    return _NC


def _make_tri_masks():
    pj = np.arange(128)[:, None]
    fi = np.arange(512)[None, :]
    blocks = [(fi >= 128 * t + pj) for t in range(4)]
    return np.concatenate(blocks, axis=1).astype(ml_dtypes.bfloat16)


def _numpy_reference(x, causal_mask, Wqkv, bqkv, Wproj, bproj):
    b, n, d = x.shape
    qkv = x @ Wqkv + bqkv
    qkv = qkv.reshape(b, n, 3, H, HD).transpose(2, 0, 3, 1, 4)
    q, k, v = qkv[0], qkv[1], qkv[2]
    s = np.einsum("bhqd,bhkd->bhqk", q, k) * (HD ** -0.5) + causal_mask
    s = s - s.max(axis=-1, keepdims=True)
    p = np.exp(s)
    p /= p.sum(axis=-1, keepdims=True)
    o = np.einsum("bhqk,bhkd->bhqd", p, v)
    o = o.transpose(0, 2, 1, 3).reshape(b, n, d)
    return (o @ Wproj + bproj).astype(np.float32)


def build_in_maps(inputs):
    x = np.asarray(inputs["x"], dtype=np.float32)
    Wqkv = np.asarray(inputs["Wqkv"], dtype=np.float32)
    bqkv = np.asarray(inputs["bqkv"], dtype=np.float32)
    Wproj = np.asarray(inputs["Wproj"], dtype=np.float32)
    bproj = np.asarray(inputs["bproj"], dtype=np.float32)
    tmask = _make_tri_masks()
    bf = ml_dtypes.bfloat16
    xTs = [np.ascontiguousarray(x[b].T.astype(bf)) for b in range(B)]
    in_maps = []
    for c in range(N_CORES):
        b, hg = c // 2, c % 2
        cs = slice(hg * C, (hg + 1) * C)
        wp_rows = Wproj[hg * C:(hg + 1) * C, :]
        bv = bqkv[2 * D + hg * C: 2 * D + (hg + 1) * C]
        bp_row = bv @ wp_rows + (bproj if hg == 0 else 0.0)
        in_maps.append({
            "xT": xTs[b],
            "wq": np.ascontiguousarray(Wqkv[:, cs].astype(bf)),
            "wk": np.ascontiguousarray(
                Wqkv[:, D + hg * C: D + (hg + 1) * C].astype(bf)),
            "wv": np.ascontiguousarray(
                Wqkv[:, 2 * D + hg * C: 2 * D + (hg + 1) * C].astype(bf)),
            "bq": np.ascontiguousarray(bqkv[cs].reshape(C, 1)),
            "wp": np.ascontiguousarray(wp_rows.astype(bf)),
            "bp": np.ascontiguousarray(
                np.broadcast_to(bp_row.astype(np.float32), (128, D))),
            "tm": tmask,
        })
    return in_maps


def kernel(x, causal_mask, Wqkv, bqkv, Wproj, bproj):
    x = np.asarray(x, dtype=np.float32)
    causal_mask = np.asarray(causal_mask, dtype=np.float32)
    Wqkv = np.asarray(Wqkv, dtype=np.float32)
    bqkv = np.asarray(bqkv, dtype=np.float32)
    Wproj = np.asarray(Wproj, dtype=np.float32)
    bproj = np.asarray(bproj, dtype=np.float32)

    # the device kernel applies causality structurally; verify the provided
    # mask is the standard causal mask and fall back to numpy if it isn't
    expected_mask = np.where(
        np.triu(np.ones((N, N), dtype=bool), k=1),
        np.float32(-1e9), np.float32(0.0))
    if causal_mask.shape != (N, N) or not np.array_equal(
            causal_mask, expected_mask):
        return _numpy_reference(x, causal_mask, Wqkv, bqkv, Wproj, bproj)

    nc = _get_nc()
    in_maps = build_in_maps(
        dict(x=x, Wqkv=Wqkv, bqkv=bqkv, Wproj=Wproj, bproj=bproj))

    res = run_bass_kernel_spmd(nc, in_maps, core_ids=list(range(N_CORES)))
    outs = [r["out"] for r in res.results]
    return np.stack([outs[2 * b] + outs[2 * b + 1] for b in range(B)], axis=0)



# revision 20
# speedup vs baseline: 1.4859x; 1.0985x over previous
"""Causal self-attention (B=4, N=2048, D=1024, H=16) on 8 trn2 NeuronCores.

Sharding: core c -> (batch b = c//2, head-group hg = c%2).  Each core runs
one batch with 8 of the 16 heads; host adds the two head-group partials.

v2 layout (all matmuls lhsT.T @ rhs, PSUM f32):
  - Stage 1 runs fully in bf16 (x^T and W tiles bf16): Q^T/K^T come out
    directly (lhsT = W, rhs = X^T), V in natural [seq, head*hd] layout.
  - Attention processes HEAD PAIRS (hh=0/1): the two K=64 score matmuls
    use PE row groups 0-63 / 64-127 (tile_position auto-derived from the
    base partitions) so they run concurrently, and both heads' scores
    land in one 2-bank [128, 1024] PSUM tile -> ONE exp instruction per
    j-tile covers both heads (halves ACT instruction overhead).
  - Exact-causal trimming: diagonal-band j-tiles compute scores / exp /
    mask / P@V only on the valid i >= j region (dual-range strided APs
    for exp/mask across the two head halves).
  - Softmax denominator rides as a ones-column in V (row 64 of the P@V
    psum).  Normalization: DVE evicts pv into the aot rows early (frees
    the PSUM banks), reciprocal per head, one K=33 "L-matrix" matmul
    broadcasts both heads' recips across partitions, and the final
    in-place muls run on GpSimd (Pool engine, otherwise idle).
  - The {0,1} causal masks multiply on DVE, restricted to the 128-wide
    diagonal block where the mask is non-trivial (beyond it, all-ones).
  - Phase software-pipelining: stage 1 of chunk ic+1 and the output
    projection of chunk ic are emitted AFTER attention(ic), so their PE
    work fills the bubbles of the ACT-bound attention inner loop.
"""

import numpy as np
import ml_dtypes

import concourse.bass as bass
import concourse.mybir as mybir
import concourse.tile as tile
from concourse import bacc
from concourse.bass_utils import run_bass_kernel_spmd

AF = mybir.ActivationFunctionType
F32 = mybir.dt.float32
F32R = mybir.dt.float32r
BF16 = mybir.dt.bfloat16

B, N, D = 4, 2048, 1024
H, HD = 16, 64
HG = 8                 # heads per core
C = HG * HD            # 512: per-core head width
NT = N // 128          # 16 seq tiles
KT = D // 128          # 8 contraction tiles of the x->qkv matmul
IC = N // 512          # 4 query chunks
NP = HG // 2           # 4 head pairs
SCALE = HD ** -0.5
N_CORES = 8


def build_nc(reps=1, skip_attn=False, variant=None):
    if variant is None:
        import os
        variant = os.environ.get("KVARIANT", "")
    nc = bacc.Bacc("TRN2", target_bir_lowering=False, debug=False,
                   num_devices=N_CORES)

    xT = nc.dram_tensor("xT", [D, N], BF16, kind="ExternalInput")
    wq = nc.dram_tensor("wq", [D, C], BF16, kind="ExternalInput")
    wk = nc.dram_tensor("wk", [D, C], BF16, kind="ExternalInput")
    wv = nc.dram_tensor("wv", [D, C], BF16, kind="ExternalInput")
    bq = nc.dram_tensor("bq", [C, 1], F32, kind="ExternalInput")
    wp = nc.dram_tensor("wp", [C, D], BF16, kind="ExternalInput")
    bp = nc.dram_tensor("bp", [128, D], F32, kind="ExternalInput")
    tm = nc.dram_tensor("tm", [128, 2048], BF16, kind="ExternalInput")
    out = nc.dram_tensor("out", [N, D], F32, kind="ExternalOutput")

    with tile.TileContext(nc) as tc:
        with (
            tc.tile_pool(name="persist", bufs=1) as persist,
            tc.tile_pool(name="xt", bufs=2) as xt_pool,
            tc.tile_pool(name="qt", bufs=2) as qt_pool,
            tc.tile_pool(name="aot", bufs=2) as aot_pool,
            tc.tile_pool(name="pt", bufs=6) as pt_pool,
            tc.tile_pool(name="ostage", bufs=4) as ostage_pool,
            tc.tile_pool(name="small", bufs=4) as small_pool,
            tc.tile_pool(name="ps_sc", bufs=(4 if variant == "seqsc" else 2),
                         space="PSUM") as ps_sc,
            tc.tile_pool(name="ps_pv", bufs=2, space="PSUM") as ps_pv,
            tc.tile_pool(name="ps_mm", bufs=2, space="PSUM") as ps_mm,
        ):
            # ---- persistent SBUF tensors ----
            # DMA priority order: first stage-1 group needs wq + x^T chunk0
            # + bq, so those go first; attention-only tensors come last.
            # One dma_start per tensor: a single InstDMACopy is split across
            # all 16 SDMA engines (near-full HBM rate) and costs one issue
            # slot on the sync queue instead of eight.
            wq_sb = persist.tile([128, KT, C], BF16)
            wk_sb = persist.tile([128, KT, C], BF16)
            wv_sb = persist.tile([128, KT, C], BF16)

            def dma_xt_chunk(ic):
                xt_t = xt_pool.tile([128, KT, 512], BF16, name="xt")
                nc.sync.dma_start(
                    out=xt_t[:, :, :],
                    in_=xT.rearrange("(t p) n -> p t n", p=128)[
                        :, :, ic * 512:(ic + 1) * 512],
                )
                return xt_t

            # the very first stage-1 group only touches wq[:, :, 0:128]
            # (ct=0), so that slice ships separately ahead of everything
            wq_r = wq.rearrange("(t p) c -> p t c", p=128)
            nc.sync.dma_start(out=wq_sb[:, :, 0:128], in_=wq_r[:, :, 0:128])
            bq_sb = persist.tile([128, C // 128], F32)
            nc.sync.dma_start(out=bq_sb[:, :],
                              in_=bq.rearrange("(t p) o -> p (t o)", p=128))
            xt_first = dma_xt_chunk(0)
            nc.sync.dma_start(out=wq_sb[:, :, 128:C], in_=wq_r[:, :, 128:C])
            nc.sync.dma_start(out=wk_sb[:, :, :],
                              in_=wk.rearrange("(t p) c -> p t c", p=128))
            nc.sync.dma_start(out=wv_sb[:, :, :],
                              in_=wv.rearrange("(t p) c -> p t c", p=128))
            tm_sb = persist.tile([128, 2048], BF16)
            nc.sync.dma_start(out=tm_sb[:, :], in_=tm[:, :])
            wp_sb = persist.tile([128, C // 128, D], BF16)
            nc.sync.dma_start(out=wp_sb[:, :, :],
                              in_=wp.rearrange("(c p) d -> p c d", p=128))
            bp_sb = persist.tile([128, D], F32)
            nc.sync.dma_start(out=bp_sb[:, :], in_=bp[:, :])



            # L-matrix for the recip partition-broadcast: out rows 0:64 copy
            # recip row 0, rows 64:128 copy recip row 32 (K=33 matmul; the
            # zero rows 1..31 contribute nothing).  Row 32 because engine
            # APs must start at 32-aligned partitions.
            l2_f = persist.tile([33, 128], F32)
            nc.vector.memset(l2_f[:, :], 0.0)
            nc.vector.memset(l2_f[0:1, 0:64], 1.0)
            nc.vector.memset(l2_f[32:33, 64:128], 1.0)
            l2_sb = persist.tile([33, 128], F32R)
            nc.vector.tensor_copy(l2_sb[:, :], l2_f[:, :])

            kt_sb = persist.tile([128, C // 128, N], BF16)   # K^T, c-major
            v_sb = persist.tile([128, NT, HG, HD + 1], BF16)  # V + ones col
            nc.gpsimd.memset(v_sb[:, :, :, HD:HD + 1], 1.0)

            def stage1(ic, xt_t):
                """QKV projection for query chunk ic."""
                qt_t = qt_pool.tile([128, C // 128, 512], BF16, name="qt")
                groups = []
                for ct in range(C // 128):
                    groups.append((
                        lambda p, ct=ct: [nc.tensor.matmul(
                            p, wq_sb[:, kt, ct * 128:(ct + 1) * 128],
                            xt_t[:, kt, :],
                            start=(kt == 0), stop=(kt == KT - 1))
                            for kt in range(KT)],
                        lambda p, ct=ct: nc.vector.tensor_scalar_add(
                            qt_t[:, ct, :], p, bq_sb[:, ct:ct + 1]),
                    ))
                for ct in range(C // 128):
                    groups.append((
                        lambda p, ct=ct: [nc.tensor.matmul(
                            p, wk_sb[:, kt, ct * 128:(ct + 1) * 128],
                            xt_t[:, kt, :],
                            start=(kt == 0), stop=(kt == KT - 1))
                            for kt in range(KT)],
                        lambda p, ct=ct: nc.vector.tensor_copy(
                            kt_sb[:, ct, ic * 512:(ic + 1) * 512], p),
                    ))
                for ntl in range(4):
                    jt = ic * 4 + ntl
                    groups.append((
                        lambda p, ntl=ntl: [nc.tensor.matmul(
                            p, xt_t[:, kt, ntl * 128:(ntl + 1) * 128],
                            wv_sb[:, kt, :],
                            start=(kt == 0), stop=(kt == KT - 1))
                            for kt in range(KT)],
                        lambda p, jt=jt: nc.vector.tensor_copy(
                            v_sb[:, jt, :, 0:HD],
                            p.rearrange("p (h w) -> p h w", w=HD)),
                    ))
                for emit_mms, evict in groups:
                    sl = ps_mm.tile([128, 512], F32, name="mm")
                    emit_mms(sl[:, :])
                    evict(sl[:, :])
                return qt_t

            def attention_pair(ic, p, qt_t, aot_t):
                """Scores+softmax+P@V for head pair p of chunk ic.
                Returns a closure that emits the normalization tail."""
                njt = 4 * ic + 4
                h0, h1 = 2 * p, 2 * p + 1
                pv0 = ps_pv.tile([HD + 1, 512], F32, name="pv")
                pv1 = ps_pv.tile([HD + 1, 512], F32, name="pv")
                pts = [None] * njt   # (pt_tile, lo) per j-tile

                def sc_stage(jt):
                    t = jt - 4 * ic          # >=0 on the diagonal band
                    lo = max(t, 0) * 128     # first valid query col
                    if variant == "seqsc":
                        # one 1-bank psum tile + dense exp + 1D mask per
                        # head half, finer sc->exp pipelining
                        pt = pt_pool.tile([128, 1024], BF16, name="pt")
                        m = tm_sb.rearrange("q (g w) -> q g w", g=4)
                        for hh in range(2):
                            ps = ps_sc.tile([128, 512], F32, name="sc")
                            nc.tensor.matmul(
                                ps[:, lo:512],
                                kt_sb[hh * 64:hh * 64 + 64, p,
                                      jt * 128:(jt + 1) * 128],
                                qt_t[hh * 64:hh * 64 + 64, p, lo:512],
                                start=True, stop=True)
                            o = 512 * hh
                            nc.scalar.activation(
                                pt[:, o + lo:o + 512], ps[:, lo:512],
                                AF.Exp, scale=SCALE)
                            if t >= 0 and variant != "nomask":
                                nc.vector.tensor_mul(
                                    pt[:, o + lo:o + lo + 128],
                                    pt[:, o + lo:o + lo + 128],
                                    m[:, t, lo:lo + 128])
                        pts[jt] = (pt, lo)
                        return
                    ps = ps_sc.tile([128, 1024], F32, name="sc")
                    nc.tensor.matmul(
                        ps[:, lo:512],
                        kt_sb[0:64, p, jt * 128:(jt + 1) * 128],
                        qt_t[0:64, p, lo:512],
                        start=True, stop=True)
                    nc.tensor.matmul(
                        ps[:, 512 + lo:1024],
                        kt_sb[64:128, p, jt * 128:(jt + 1) * 128],
                        qt_t[64:128, p, lo:512],
                        start=True, stop=True)
                    pt = pt_pool.tile([128, 1024], BF16, name="pt")
                    if variant == "splitexp":
                        # dense 1D APs, one exp per head half
                        nc.scalar.activation(pt[:, lo:512], ps[:, lo:512],
                                             AF.Exp, scale=SCALE)
                        nc.scalar.activation(pt[:, 512 + lo:1024],
                                             ps[:, 512 + lo:1024],
                                             AF.Exp, scale=SCALE)
                    elif lo:
                        ap_in = ps.rearrange(
                            "q (g w) -> q g w", g=2)[:, :, lo:512]
                        ap_out = pt.rearrange(
                            "q (g w) -> q g w", g=2)[:, :, lo:512]
                        nc.scalar.activation(ap_out, ap_in, AF.Exp,
                                             scale=SCALE)
                    else:
                        nc.scalar.activation(pt[:, :], ps[:, :], AF.Exp,
                                             scale=SCALE)
                    if t >= 0 and variant != "nomask":
                        # only the 128-wide diagonal block i in
                        # [128t, 128t+128) has non-trivial mask values;
                        # everything beyond it is all-ones
                        m = tm_sb.rearrange("q (g w) -> q g w", g=4)
                        eng = nc.gpsimd if variant == "gmask" else nc.vector
                        if variant == "splitexp":
                            eng.tensor_mul(
                                pt[:, lo:lo + 128], pt[:, lo:lo + 128],
                                m[:, t, lo:lo + 128])
                            eng.tensor_mul(
                                pt[:, 512 + lo:512 + lo + 128],
                                pt[:, 512 + lo:512 + lo + 128],
                                m[:, t, lo:lo + 128])
                        else:
                            d_out = pt.rearrange(
                                "q (g w) -> q g w", g=2)[:, :, lo:lo + 128]
                            eng.tensor_mul(
                                d_out, d_out,
                                m[:, t:t + 1, lo:lo + 128].broadcast_to(
                                    (128, 2, 128)))
                    pts[jt] = (pt, lo)

                def pv_stage(jt):
                    pt, lo = pts[jt]
                    nc.tensor.matmul(
                        pv0[:, lo:512], v_sb[:, jt, h0, :], pt[:, lo:512],
                        start=(jt == 0), stop=(jt == njt - 1))
                    nc.tensor.matmul(
                        pv1[:, lo:512], v_sb[:, jt, h1, :],
                        pt[:, 512 + lo:1024],
                        start=(jt == 0), stop=(jt == njt - 1))
                    pts[jt] = None

                LAG = 3
                for jt in range(njt):
                    sc_stage(jt)
                    if jt >= LAG:
                        pv_stage(jt - LAG)
                for jt in range(max(0, njt - LAG), njt):
                    pv_stage(jt)

                # evict unnormalized attn-out straight into the aot rows
                # (DVE allows the 0->64 partition shift) and take the two
                # denominator recips; pv banks free right after
                den = small_pool.tile([1, 1024], F32, name="den",
                                      tag="den", bufs=3)
                rcf = small_pool.tile([1, 1024], F32, name="rcf",
                                      tag="rcf", bufs=3)
                if variant != "noepi":
                    nc.vector.tensor_copy(den[0:1, 0:512],
                                          pv0[HD:HD + 1, :])
                    nc.vector.tensor_copy(den[0:1, 512:1024],
                                          pv1[HD:HD + 1, :])
                nc.vector.tensor_copy(aot_t[0:64, p, :], pv0[0:HD, :])
                nc.vector.tensor_copy(aot_t[64:128, p, :], pv1[0:HD, :])
                if variant != "noepi":
                    # ~5x faster than nc.vector.reciprocal; denominators are
                    # in [~1e-2, ~1e6] so the approx edge cases can't hit,
                    # and ~18 correct bits is far beyond what we need.
                    # The custom DVE op needs SBUF in/out at partition 0
                    # with plain-copy staging around it: cross-engine deps
                    # on its APs are unreliable (silent no-op / garbage),
                    # so every producer/consumer it touches stays on DVE
                    # where FIFO order protects it.
                    nc.vector.reciprocal_approx_fast(
                        out=rcf[0:1, :], in_=den[0:1, :])

                rc2 = small_pool.tile([33, 512], F32R, name="rc",
                                      tag="rc", bufs=3)
                if variant != "noepi":
                    with nc.allow_low_precision(reason="f32r recip ok"):
                        nc.vector.tensor_copy(rc2[0:1, :], rcf[0:1, 0:512])
                        nc.vector.tensor_copy(rc2[32:33, :],
                                              rcf[0:1, 512:1024])

                def tail():
                    if variant == "noepi":
                        return
                    rep = ps_pv.tile([128, 512], F32, name="pv")
                    nc.tensor.matmul(rep[:, :], l2_sb[:, :], rc2[:, :],
                                     start=True, stop=True)
                    rep_sb = small_pool.tile([128, 512], BF16, name="repsb",
                                             tag="repsb", bufs=2)
                    nc.vector.tensor_copy(rep_sb[:, :], rep[:, :])
                    eng = nc.vector if variant == "dvemul" else nc.gpsimd
                    eng.tensor_mul(
                        aot_t[:, p, :], aot_t[:, p, :], rep_sb[:, :])
                return tail

            def attention(ic, qt_t):
                aot_t = aot_pool.tile([128, C // 128, 512], BF16, name="aot")
                pending = None
                for p in range(0 if skip_attn else NP):
                    t = attention_pair(ic, p, qt_t, aot_t)
                    if pending is not None:
                        pending()
                    pending = t
                if pending is not None:
                    pending()
                return aot_t

            def proj(ic, aot_t):
                for ntl in range(4):
                    nt = ic * 4 + ntl
                    pss = [ps_mm.tile([128, 512], F32, name="mm")
                           for _ in range(2)]
                    for ct in range(C // 128):
                        for dc in range(2):
                            nc.tensor.matmul(
                                pss[dc][:, :],
                                aot_t[:, ct, ntl * 128:(ntl + 1) * 128],
                                wp_sb[:, ct, dc * 512:(dc + 1) * 512],
                                start=(ct == 0), stop=(ct == C // 128 - 1),
                            )
                    ot = ostage_pool.tile([128, 1024], F32, name="ot")
                    for dc in range(2):
                        nc.vector.tensor_add(
                            ot[:, dc * 512:(dc + 1) * 512], pss[dc][:, :],
                            bp_sb[:, dc * 512:(dc + 1) * 512])
                    nc.sync.dma_start(
                        out=out[nt * 128:(nt + 1) * 128, :], in_=ot[:, :])

            for _rep in range(reps):
                qt = None
                xt_next = xt_first if _rep == 0 else dma_xt_chunk(0)
                for ic in range(IC):
                    if ic == 0:
                        qt = stage1(0, xt_next)
                    aot_t = attention(ic, qt)
                    if ic + 1 < IC:
                        xt_next = dma_xt_chunk(ic + 1)
                        qt = stage1(ic + 1, xt_next)
                    proj(ic, aot_t)

    nc.compile()
    return nc


_NC = None


def _get_nc():
    global _NC
    if _NC is None:
        _NC = build_nc()
    return _NC


def _make_tri_masks():
    pj = np.arange(128)[:, None]
    fi = np.arange(512)[None, :]
    blocks = [(fi >= 128 * t + pj) for t in range(4)]
    return np.concatenate(blocks, axis=1).astype(ml_dtypes.bfloat16)


def _numpy_reference(x, causal_mask, Wqkv, bqkv, Wproj, bproj):
    b, n, d = x.shape
    qkv = x @ Wqkv + bqkv
    qkv = qkv.reshape(b, n, 3, H, HD).transpose(2, 0, 3, 1, 4)
    q, k, v = qkv[0], qkv[1], qkv[2]
    s = np.einsum("bhqd,bhkd->bhqk", q, k) * (HD ** -0.5) + causal_mask
    s = s - s.max(axis=-1, keepdims=True)
    p = np.exp(s)
    p /= p.sum(axis=-1, keepdims=True)
    o = np.einsum("bhqk,bhkd->bhqd", p, v)
    o = o.transpose(0, 2, 1, 3).reshape(b, n, d)
    return (o @ Wproj + bproj).astype(np.float32)


def build_in_maps(inputs):
    x = np.asarray(inputs["x"], dtype=np.float32)
    Wqkv = np.asarray(inputs["Wqkv"], dtype=np.float32)
    bqkv = np.asarray(inputs["bqkv"], dtype=np.float32)
    Wproj = np.asarray(inputs["Wproj"], dtype=np.float32)
    bproj = np.asarray(inputs["bproj"], dtype=np.float32)
    tmask = _make_tri_masks()
    bf = ml_dtypes.bfloat16
    xTs = [np.ascontiguousarray(x[b].T.astype(bf)) for b in range(B)]
    in_maps = []
    for c in range(N_CORES):
        b, hg = c // 2, c % 2
        cs = slice(hg * C, (hg + 1) * C)
        wp_rows = Wproj[hg * C:(hg + 1) * C, :]
        bv = bqkv[2 * D + hg * C: 2 * D + (hg + 1) * C]
        bp_row = bv @ wp_rows + (bproj if hg == 0 else 0.0)
        in_maps.append({
            "xT": xTs[b],
            "wq": np.ascontiguousarray(Wqkv[:, cs].astype(bf)),
            "wk": np.ascontiguousarray(
                Wqkv[:, D + hg * C: D + (hg + 1) * C].astype(bf)),
            "wv": np.ascontiguousarray(
                Wqkv[:, 2 * D + hg * C: 2 * D + (hg + 1) * C].astype(bf)),
            "bq": np.ascontiguousarray(bqkv[cs].reshape(C, 1)),
            "wp": np.ascontiguousarray(wp_rows.astype(bf)),
            "bp": np.ascontiguousarray(
                np.broadcast_to(bp_row.astype(np.float32), (128, D))),
            "tm": tmask,
        })
    return in_maps


def kernel(x, causal_mask, Wqkv, bqkv, Wproj, bproj):
    x = np.asarray(x, dtype=np.float32)
    causal_mask = np.asarray(causal_mask, dtype=np.float32)
    Wqkv = np.asarray(Wqkv, dtype=np.float32)
    bqkv = np.asarray(bqkv, dtype=np.float32)
    Wproj = np.asarray(Wproj, dtype=np.float32)
    bproj = np.asarray(bproj, dtype=np.float32)

    # the device kernel applies causality structurally; verify the provided
    # mask is the standard causal mask and fall back to numpy if it isn't
    expected_mask = np.where(
        np.triu(np.ones((N, N), dtype=bool), k=1),
        np.float32(-1e9), np.float32(0.0))
    if causal_mask.shape != (N, N) or not np.array_equal(
            causal_mask, expected_mask):
        return _numpy_reference(x, causal_mask, Wqkv, bqkv, Wproj, bproj)

    nc = _get_nc()
    in_maps = build_in_maps(
        dict(x=x, Wqkv=Wqkv, bqkv=bqkv, Wproj=Wproj, bproj=bproj))

    res = run_bass_kernel_spmd(nc, in_maps, core_ids=list(range(N_CORES)))
    outs = [r["out"] for r in res.results]
    return np.stack([outs[2 * b] + outs[2 * b + 1] for b in range(B)], axis=0)

